# revision 2
# baseline (speedup 1.0000x reference)
"""Trainium2 Bass kernel for ColorEntropyLoss.

Math (per batch b, attention map s):
    color_dist[b,s,c] = sum_h attn[b,s,h] * (grid[b,h] == c)       # 10-bin weighted histogram
    p = color_dist / (sum_c color_dist + 1e-8)
    entropy[b,s]      = -sum_c p * log(p + 1e-8)
    out               = mean(entropy)

Sharding: pure data parallelism over batch B=512 across 8 NeuronCores
(64 batches/core), 8 groups of 8 batches per core; a group packs 128
SBUF partitions as (8 batches x 16 maps).

Layout: the host hands attn already in "contraction-major" order —
per group a [128, 4096] f32 slab whose partition dim is pixel-in-chunk
and whose columns are chunk-major x (b,s) — so the histogram is a
straight PSUM-accumulated bf16 matmul chain with NO on-device
transposes:
    out[(b,s),(c,b')] += attnT_chunk.T @ onehot_chunk
The grid arrives host-transposed as bf16 [128, 32*64]; one broadcast
is_equal per group builds all 32 one-hot chunks. attn reaches SBUF as
bf16 via SWDGE cast-DMA (f32 HBM read, bf16 SBUF write); group 0's
first half goes over the two HWDGE queues (sync + scalar) in f32 with
on-chip casts to cover the ~4.5us SWDGE ring-arm latency. The final
mean over the 8192 per-(b,s) entropies is done host-side (the "cheap
all-reduce" from the sharding hint).
"""

import numpy as np
from contextlib import ExitStack

NUM_COLORS = 10
EPS = 1e-8
B, S, H, W = 512, 16, 64, 64
HW = H * W                      # 4096
N_CORES = 8
B_PER_CORE = B // N_CORES       # 64
N_GROUPS = 8                    # groups per core
B_PER_GROUP = B_PER_CORE // N_GROUPS  # 8 batches -> 128 partitions
P = 128
CHUNK = 128
N_CHUNKS = HW // CHUNK          # 32
NC80 = B_PER_GROUP * NUM_COLORS  # 80
QTR = HW // 4                   # 1024 cols = 8 chunks

_CACHE = {}


def _build_nc():
    import concourse.bacc as bacc
    import concourse.tile as tile
    import concourse.bass as bass
    from concourse import mybir

    f32 = mybir.dt.float32
    bf16 = mybir.dt.bfloat16
    OP = mybir.AluOpType
    AF = mybir.ActivationFunctionType
    AX = mybir.AxisListType

    nc = bacc.Bacc(
        "TRN2", target_bir_lowering=False, debug=False, num_devices=N_CORES
    )

    # row = g*128 + p (p = pixel-in-chunk), col = k*128 + b'*16 + s
    attn_in = nc.dram_tensor(
        "attn_in", [N_GROUPS * P, HW], f32, kind="ExternalInput"
    ).ap()
    # row = p, col = k*64 + b   (b = batch within core)
    grid_in = nc.dram_tensor(
        "grid_in", [P, N_CHUNKS * B_PER_CORE], bf16, kind="ExternalInput"
    ).ap()
    ent_out = nc.dram_tensor(
        "ent_out", [P, N_GROUPS], f32, kind="ExternalOutput"
    ).ap()

    with tile.TileContext(nc) as tc:
        with ExitStack() as ctx:
            singles = ctx.enter_context(tc.tile_pool(name="singles", bufs=1))
            pool_s = ctx.enter_context(tc.tile_pool(name="pool_s", bufs=3))
            psum_cd = ctx.enter_context(
                tc.tile_pool(name="psum_cd", bufs=3, space="PSUM")
            )

            # ---- HWDGE first: gridT (bf16, 512KB) on the sync queue,
            # then group 0's first half in f32 across both HWDGE queues
            # (sync + scalar) while the SWDGE ring arms. ----
            gridT = singles.tile([P, N_CHUNKS * B_PER_CORE], bf16)
            nc.sync.dma_start(out=gridT, in_=grid_in)

            attn_f32 = singles.tile([P, 2 * QTR], f32)  # g0 q0,q1 staging
            nc.sync.dma_start(
                out=attn_f32[:, 0:QTR], in_=attn_in[0:P, 0:QTR]
            )
            nc.scalar.dma_start(
                out=attn_f32[:, QTR : 2 * QTR],
                in_=attn_in[0:P, QTR : 2 * QTR],
            )

            # ---- SWDGE cast-DMA burst: descriptor-gen first thing on
            # gpsimd. g0's second half first (quarters), then g1..g6
            # whole, then g7 in quarters so the tail drains fast. ----
            attn_tiles = [
                singles.tile([P, HW], bf16, name=f"attnT{g}")
                for g in range(N_GROUPS)
            ]
            nc.gpsimd.dma_start(
                out=attn_tiles[0][:, 2 * QTR : 3 * QTR],
                in_=attn_in[0:P, 2 * QTR : 3 * QTR],
            )
            nc.gpsimd.dma_start(
                out=attn_tiles[0][:, 3 * QTR : 4 * QTR],
                in_=attn_in[0:P, 3 * QTR : 4 * QTR],
            )
            # const_cb[p, c*8+b] = c (0..9, exact in bf16) -- needed by
            # the first is_equal, so emit between early desc-gens.
            const_cb = singles.tile([P, NC80], bf16)
            nc.gpsimd.iota(
                const_cb,
                pattern=[[1, NUM_COLORS], [0, B_PER_GROUP]],
                base=0,
                channel_multiplier=0,
                allow_small_or_imprecise_dtypes=True,
            )
            for g in range(1, N_GROUPS - 1):
                nc.gpsimd.dma_start(
                    out=attn_tiles[g], in_=attn_in[g * P : (g + 1) * P, :]
                )
            for q in range(4):
                nc.gpsimd.dma_start(
                    out=attn_tiles[7][:, q * QTR : (q + 1) * QTR],
                    in_=attn_in[7 * P : 8 * P, q * QTR : (q + 1) * QTR],
                )

            ent_sb = singles.tile([P, N_GROUPS], f32)
            eps_tile = singles.tile([P, 1], f32)
            nc.vector.memset(eps_tile, EPS)

            # Block-diagonal selector [128, 80] in (c, b) layout: row
            # p=(b,s) keeps cols c*8 + (p//16).
            mask_bd = singles.tile([P, NC80], f32)
            nc.vector.memset(mask_bd, 1.0)
            nc.gpsimd.affine_select(
                out=mask_bd,
                in_=mask_bd,
                compare_op=OP.is_ge,
                fill=0.0,
                base=0,
                pattern=[[0, NUM_COLORS], [-S, B_PER_GROUP]],
                channel_multiplier=1,
            )
            nc.gpsimd.affine_select(
                out=mask_bd,
                in_=mask_bd,
                compare_op=OP.is_ge,
                fill=0.0,
                base=S - 1,
                pattern=[[0, NUM_COLORS], [S, B_PER_GROUP]],
                channel_multiplier=-1,
            )

            # ---- on-chip casts for g0's HWDGE half (f32 -> bf16) ----
            nc.vector.tensor_copy(
                attn_tiles[0][:, 0:QTR], attn_f32[:, 0:QTR]
            )
            nc.scalar.copy(
                out=attn_tiles[0][:, QTR : 2 * QTR],
                in_=attn_f32[:, QTR : 2 * QTR],
            )

            for g in range(N_GROUPS):
                attnT = attn_tiles[g]

                # ---- one-hot masks for all 32 chunks in one is_equal ----
                # mask flat [128, 2560]: col = k*80 + c*8 + b
                mask = singles.tile(
                    [P, N_CHUNKS * NC80], bf16, name=f"mask{g}"
                )
                gT = gridT[:, :]
                in0 = bass.AP(
                    tensor=gT.tensor,
                    offset=gT.offset + g * B_PER_GROUP,
                    ap=[
                        gT.ap[0],
                        [B_PER_CORE, N_CHUNKS],
                        [0, NUM_COLORS],
                        [1, B_PER_GROUP],
                    ],
                )
                cC = const_cb[:, :]
                in1 = bass.AP(
                    tensor=cC.tensor,
                    offset=cC.offset,
                    ap=[cC.ap[0], [0, N_CHUNKS], [1, NC80]],
                )
                mk = mask[:, :]
                mout = bass.AP(
                    tensor=mk.tensor,
                    offset=mk.offset,
                    ap=[mk.ap[0], [NC80, N_CHUNKS], [1, NC80]],
                )
                nc.vector.tensor_tensor(
                    out=mout, in0=in0, in1=in1, op=OP.is_equal
                )

                # ---- histogram: 32 accumulating bf16 matmuls -> PSUM f32 ----
                ps_c = psum_cd.tile([P, NC80], f32, name="ps_c", tag="cd")
                for k in range(N_CHUNKS):
                    nc.tensor.matmul(
                        ps_c,
                        attnT[:, k * CHUNK : (k + 1) * CHUNK],
                        mask[:, k * NC80 : (k + 1) * NC80],
                        start=(k == 0),
                        stop=(k == N_CHUNKS - 1),
                    )

                # ---- masked copy to SBUF + row-sum in one op ----
                cd = pool_s.tile([P, NC80], f32, name="cd_sb", tag="cd_sb")
                ssum = pool_s.tile([P, 1], f32, name="ssum", tag="ssum")
                nc.vector.scalar_tensor_tensor(
                    out=cd,
                    in0=ps_c,
                    scalar=1.0,
                    in1=mask_bd[:, :],
                    op0=OP.mult,
                    op1=OP.mult,
                    accum_out=ssum,
                )

                # ---- entropy per (b,s) row over all 80 cols (zeros inert) ----
                nc.vector.tensor_scalar_add(ssum, ssum, EPS)
                srec = pool_s.tile([P, 1], f32, name="srec", tag="srec")
                nc.vector.reciprocal(srec, ssum)
                p_t = pool_s.tile([P, NC80], f32, name="p_t", tag="p_t")
                nc.vector.tensor_scalar_mul(p_t, cd, srec[:, :])
                lp = pool_s.tile([P, NC80], f32, name="lp", tag="lp")
                nc.scalar.activation(lp, p_t, AF.Ln, bias=eps_tile[:, :])
                q = pool_s.tile([P, NC80], f32, name="q", tag="q")
                nc.vector.tensor_mul(q, p_t, lp)
                # ent_sb holds +sum(p*ln(p+eps)); host negates.
                nc.vector.reduce_sum(ent_sb[:, g : g + 1], q, axis=AX.X)

            nc.sync.dma_start(out=ent_out, in_=ent_sb)

    nc.compile()
    return nc


def _get_nc():
    if "nc" not in _CACHE:
        _CACHE["nc"] = _build_nc()
    return _CACHE["nc"]


def _make_in_maps(attn_weights, grids):
    import ml_dtypes

    attn = np.ascontiguousarray(attn_weights, dtype=np.float32).reshape(
        B, S, HW
    )
    grid = np.asarray(grids)
    in_maps = []
    for c in range(N_CORES):
        lo, hi = c * B_PER_CORE, (c + 1) * B_PER_CORE
        # [64,16,4096] -> [g, b', s, k, p] -> [g, p, k, b', s]
        a5 = attn[lo:hi].reshape(N_GROUPS, B_PER_GROUP, S, N_CHUNKS, CHUNK)
        a_t = np.ascontiguousarray(a5.transpose(0, 4, 3, 1, 2)).reshape(
            N_GROUPS * P, HW
        )
        # [64,4096] -> [b, k, p] -> [p, k, b], values 0..9 exact in bf16
        g3 = grid[lo:hi].reshape(B_PER_CORE, N_CHUNKS, CHUNK)
        g_t = (
            np.ascontiguousarray(g3.transpose(2, 1, 0))
            .astype(np.float32)
            .astype(ml_dtypes.bfloat16)
            .reshape(P, N_CHUNKS * B_PER_CORE)
        )
        in_maps.append({"attn_in": a_t, "grid_in": g_t})
    return in_maps


def kernel(attn_weights: np.ndarray, grids: np.ndarray) -> np.ndarray:
    from concourse.bass_utils import run_bass_kernel_spmd

    nc = _get_nc()
    in_maps = _make_in_maps(attn_weights, grids)
    res = run_bass_kernel_spmd(nc, in_maps, core_ids=list(range(N_CORES)))

    total = 0.0
    for c in range(N_CORES):
        total += float(res.results[c]["ent_out"].astype(np.float64).sum())
    return np.float32(-total / (B * S))
